# revision 11
# baseline (speedup 1.0000x reference)
"""Trainium2 Bass kernel for nn_MultiHeadCDGCN (v2, all-bf16 matmuls).

Math (per batch b):
  t_w  = softmax(x, axis=T);  TAtt = sum_T(x * t_w)          [N, D]
  Q    = x @ W_Q.T                                           [T, N, D]
  K    = TAtt @ W_K.T ; V = TAtt @ W_V.T                     [N, D]
  S_th = Q_th @ K_h.T / sqrt(dh)   (per t, head h)           [N, N]
  out  = (relu(S) + I) @ V = relu(S) @ V + V                 [T, N, D]

Sharding: data-parallel over B across 8 NeuronCores (B == 8, one batch
per core); no collectives.

v2 structure (vs v1):
  - All matmuls in bf16 (HW: S 0.61x, AV 0.52x, Q 0.31x of fp32 time;
    total rel err ~1e-2 budget is 2e-2).
  - GPSIMD cannot touch PSUM on TRN2, so every PSUM evacuation is on
    ACT/DVE; GPSIMD gets the SBUF-only sum_e accumulation, while the
    sum_xe accumulation runs on the PE as identity-matmul PSUM
    accumulation.
  - The +V identity term is folded into the A@V PSUM group by
    prefilling po with V^T (identity matmul), so the po evacuation is
    a plain copy.
  - Output is PE-transposed back to natural [tn, d] layout and DMAd
    as contiguous 1KB-per-row descriptors (v1 wrote 128B descriptors
    through ~185us of sequencer time).
"""

import sys

import numpy as np

sys.path.insert(0, "/opt/trn_rl_repo")

import ml_dtypes  # noqa: E402

import concourse.bacc as bacc  # noqa: E402
import concourse.tile as tile  # noqa: E402
from concourse import mybir  # noqa: E402
from concourse.masks import make_identity  # noqa: E402
from concourse.bass_utils import run_bass_kernel_spmd  # noqa: E402

F32 = mybir.dt.float32
BF16 = mybir.dt.bfloat16
AF = mybir.ActivationFunctionType
ALU = mybir.AluOpType

B, T, N, D, H, DH = 8, 32, 256, 256, 8, 32
P = 128
NCHUNKS = 16  # tn chunks of 512 (2 frames each)
CHUNK_T = 2
CHUNK_TN = CHUNK_T * N  # 512

_CACHE: dict = {}


def _build_program():
    nc = bacc.Bacc()

    x_d = nc.dram_tensor("x", [T, N, D], F32, kind="ExternalInput")
    wqt_d = nc.dram_tensor("wqt", [D, D], BF16, kind="ExternalInput")
    wkt_d = nc.dram_tensor("wkt", [D, D], BF16, kind="ExternalInput")
    wvt_d = nc.dram_tensor("wvt", [D, D], BF16, kind="ExternalInput")
    out_d = nc.dram_tensor("out", [T, N, D], F32, kind="ExternalOutput")

    with tile.TileContext(nc) as tc:
        with (
            tc.tile_pool(name="consts", bufs=1) as consts,
            tc.tile_pool(name="xa", bufs=3) as xa_pool,
            tc.tile_pool(name="xt", bufs=3) as xt_pool,
            tc.tile_pool(name="ew", bufs=6) as e_pool,
            tc.tile_pool(name="at", bufs=16) as a_pool,
            tc.tile_pool(name="ot", bufs=4) as o_pool,
            tc.tile_pool(name="misc", bufs=1) as misc,
        ):
            eye = consts.tile([P, P], F32)
            make_identity(nc, eye)
            eye_bf = consts.tile([P, P], BF16)
            nc.vector.tensor_copy(eye_bf, eye)

            # Weights [k, j], k split over 2 partition tiles, bf16.
            wqt_sb = consts.tile([P, 2, D], BF16)
            wkt_sb = consts.tile([P, 2, D], BF16)
            wvt_sb = consts.tile([P, 2, D], BF16)
            for w_sb, w_d in ((wqt_sb, wqt_d), (wkt_sb, wkt_d), (wvt_sb, wvt_d)):
                for kc in range(2):
                    nc.sync.dma_start(
                        out=w_sb[:, kc, :],
                        in_=w_d[kc * P : (kc + 1) * P, :],
                    )

            # Q.T strip [j, tn] resident, bf16 (j split over 2 tiles).
            qt_sb = consts.tile([P, 2, T * N], F32)

            # sum_e wide accumulator (SBUF, GPSIMD-owned).
            acc_e = consts.tile([P, 2, CHUNK_TN], F32)
            nc.gpsimd.memset(acc_e, 0.0)

            # ============ Phase A + B (stat PSUM pool scoped) ============
            with (
                tc.tile_pool(name="ps_t", bufs=3, space="PSUM") as ps_t,
                tc.tile_pool(name="ps_q", bufs=2, space="PSUM") as ps_q,
                tc.tile_pool(name="ps_s", bufs=1, space="PSUM") as ps_s,
            ):
                # sum_xe accumulators (PSUM, PE identity-matmul accumulation).
                # One full bank per dc so the two long-lived accumulation
                # groups never share a bank.
                acc_xe_t = [
                    ps_s.tile([P, CHUNK_TN], F32, name=f"accxe{dc}") for dc in range(2)
                ]
                acc_xe = {dc: acc_xe_t[dc][:, 0:N] for dc in range(2)}

                for c in range(NCHUNKS):
                    t0 = c * CHUNK_T
                    xa = xa_pool.tile([P, 4, D], F32)
                    nc.sync.dma_start(
                        out=xa,
                        in_=x_d[t0 : t0 + CHUNK_T].rearrange(
                            "t (s p) d -> p (t s) d", p=P
                        ),
                    )

                    xt = xt_pool.tile([P, 2, CHUNK_TN], BF16)
                    for dc in range(2):
                        pt = ps_t.tile([P, CHUNK_TN], F32, tag="pt", name=f"pt{dc}")
                        for s in range(4):
                            nc.tensor.transpose(
                                pt[:, s * P : (s + 1) * P],
                                xa[:, s, dc * P : (dc + 1) * P],
                                eye,
                            )
                        # x.T chunk (bf16) for the Q matmul + xe product.
                        nc.scalar.activation(xt[:, dc, :], pt, AF.Copy)
                        e_t = e_pool.tile([P, CHUNK_TN], BF16, name="e")
                        nc.scalar.activation(e_t, pt, AF.Exp)
                        # xe in bf16 from SBUF operands (DVE 2x mode).
                        xe_t = e_pool.tile([P, CHUNK_TN], BF16, name="xe")
                        nc.vector.tensor_mul(xe_t, xt[:, dc, :], e_t)
                        # sum_e += e_t on GPSIMD (SBUF only).
                        nc.gpsimd.tensor_add(acc_e[:, dc, :], acc_e[:, dc, :], e_t)
                        # sum_xe += xe_t on PE (identity matmul accumulation).
                        for ti in range(CHUNK_T):
                            first = c == 0 and ti == 0
                            last = c == NCHUNKS - 1 and ti == CHUNK_T - 1
                            nc.tensor.matmul(
                                acc_xe[dc],
                                eye_bf,
                                xe_t[:, ti * N : (ti + 1) * N],
                                start=first,
                                stop=last,
                                skip_group_check=True,
                            )

                    # Q.T chunk: [j, tn] = sum_k W_Q.T[k, j]^T x.T[k, tn]
                    for jc in range(2):
                        pq = ps_q.tile([P, CHUNK_TN], F32, tag="pq", name=f"pq{jc}")
                        for kc in range(2):
                            nc.tensor.matmul(
                                pq,
                                wqt_sb[:, kc, jc * P : (jc + 1) * P],
                                xt[:, kc, :],
                                start=(kc == 0),
                                stop=(kc == 1),
                            )
                        dst = qt_sb[:, jc, c * CHUNK_TN : (c + 1) * CHUNK_TN]
                        if jc == 0:
                            nc.scalar.activation(dst, pq, AF.Copy)
                        else:
                            nc.vector.tensor_copy(dst, pq)

                # ---------------- Phase B: TAtt.T, K.T, V.T, V
                sum_e = misc.tile([P, 2, N], F32)
                for dc in range(2):
                    nc.vector.tensor_add(
                        sum_e[:, dc, :],
                        acc_e[:, dc, 0:N],
                        acc_e[:, dc, N : 2 * N],
                    )
                rec = misc.tile([P, 2, N], F32)
                tatt_t = misc.tile([P, 2, N], BF16)  # TAtt.T [d, n] bf16
                for dc in range(2):
                    nc.vector.reciprocal(rec[:, dc, :], sum_e[:, dc, :])
                    nc.vector.scalar_tensor_tensor(
                        out=tatt_t[:, dc, :],
                        in0=acc_xe[dc],
                        scalar=1.0,
                        in1=rec[:, dc, :],
                        op0=ALU.mult,
                        op1=ALU.mult,
                    )

                kt_sb = consts.tile([P, 2, N], F32)  # K.T [j, m] (pre-scaled)
                vt_sb = consts.tile([P, 2, N], F32)  # V.T [j, m]
                for w_sb, o_sb2 in ((wkt_sb, kt_sb), (wvt_sb, vt_sb)):
                    for jc in range(2):
                        pk = ps_q.tile([P, N], F32, tag="pq", name="pk")
                        for kc in range(2):
                            nc.tensor.matmul(
                                pk,
                                w_sb[:, kc, jc * P : (jc + 1) * P],
                                tatt_t[:, kc, :],
                                start=(kc == 0),
                                stop=(kc == 1),
                            )
                        nc.vector.tensor_copy(o_sb2[:, jc, :], pk)

                v_sb = consts.tile([P, 2, D], F32)  # V [m, j]
                for mc in range(2):
                    pv = ps_q.tile([P, D], F32, tag="pq", name="pv")
                    for kc in range(2):
                        nc.tensor.matmul(
                            pv,
                            tatt_t[:, kc, mc * P : (mc + 1) * P],
                            wvt_sb[:, kc, :],
                            start=(kc == 0),
                            stop=(kc == 1),
                        )
                    nc.scalar.activation(v_sb[:, mc, :], pv, AF.Copy)

                # V.T doubled over the 2 frames of a chunk, per head group.
                vt2 = consts.tile([P, 2, 2, N], F32)
                for hg in range(2):
                    for ti in range(2):
                        nc.gpsimd.tensor_copy(vt2[:, hg, ti, :], vt_sb[:, hg, :])

            # ============ Phase C: attention + output ============
            with (
                tc.tile_pool(name="ps_a", bufs=2, space="PSUM") as ps_a,
                tc.tile_pool(name="ps_o", bufs=2, space="PSUM") as ps_o,
                tc.tile_pool(name="ps_n", bufs=2, space="PSUM") as ps_n,
            ):
                # 5 ACT : 3 DVE relu-evac split (po/onat ride on DVE/both).
                relu_acts = (0, 2, 4, 6, 7)

                def s_block(c):
                    """S matmuls + relu evac for chunk c. Returns a_str."""
                    a_str = {}
                    k = 0
                    for hg in range(2):
                        for mc in range(2):
                            for rp in range(2):  # head pairs share a 2-bank tile
                                ps2 = ps_a.tile(
                                    [P, 2 * CHUNK_TN],
                                    F32,
                                    tag="psa",
                                    name=f"ps{hg}{mc}{rp}",
                                )
                                for rh in range(2):
                                    r = rp * 2 + rh
                                    nc.tensor.matmul(
                                        ps2[:, rh * CHUNK_TN : (rh + 1) * CHUNK_TN],
                                        kt_sb[
                                            r * 32 : (r + 1) * 32,
                                            hg,
                                            mc * P : (mc + 1) * P,
                                        ],
                                        qt_sb[
                                            r * 32 : (r + 1) * 32,
                                            hg,
                                            c * CHUNK_TN : (c + 1) * CHUNK_TN,
                                        ],
                                        start=True,
                                        stop=True,
                                        tile_position=(r * 32, 0),
                                    )
                                a2 = a_pool.tile(
                                    [P, 2 * CHUNK_TN],
                                    F32,
                                    tag="at",
                                    name=f"a{hg}{mc}{rp}",
                                )
                                if k in relu_acts:
                                    nc.scalar.activation(a2, ps2, AF.Relu)
                                else:
                                    nc.vector.tensor_scalar_max(a2, ps2, 0.0)
                                k += 1
                                for rh in range(2):
                                    a_str[(hg, rp * 2 + rh, mc)] = a2[
                                        :, rh * CHUNK_TN : (rh + 1) * CHUNK_TN
                                    ]
                    return a_str

                def av_block(c, a_str):
                    """Prefill po with V^T, accumulate A@V, evac. Returns o_sbs."""
                    o_sbs = []
                    for hg in range(2):
                        po = ps_o.tile([P, CHUNK_TN], F32, tag="po", name=f"po{hg}")
                        for mc in range(2):
                            for r in range(4):
                                h = hg * 4 + r
                                nc.tensor.matmul(
                                    po[r * 32 : (r + 1) * 32, :],
                                    v_sb[:, mc, h * 32 : (h + 1) * 32],
                                    a_str[(hg, r, mc)],
                                    start=(mc == 0),
                                    stop=(mc == 1),
                                    tile_position=(0, r * 32),
                                    skip_group_check=True,
                                )
                        o_sb = o_pool.tile([P, CHUNK_TN], F32, name=f"o{hg}")
                        nc.vector.scalar_tensor_tensor(
                            out=o_sb,
                            in0=po,
                            scalar=1.0,
                            in1=vt2[:, hg, :, :],
                            op0=ALU.mult,
                            op1=ALU.add,
                        )
                        o_sbs.append(o_sb)
                    return o_sbs

                def out_block(c, o_sbs):
                    """PE-transpose to natural layout, evac, DMA per frame."""
                    t0 = c * CHUNK_T
                    for ti in range(CHUNK_T):
                        pn = ps_n.tile([P, 2, D], F32, tag="pn", name=f"pn{ti}")
                        for nh in range(2):
                            for hg in range(2):
                                nc.tensor.transpose(
                                    pn[:, nh, hg * P : (hg + 1) * P],
                                    o_sbs[hg][
                                        :, ti * N + nh * P : ti * N + (nh + 1) * P
                                    ],
                                    eye,
                                )
                        o_nat = o_pool.tile([P, 2, D], F32, tag="onat", name=f"on{ti}")
                        if ti == 0:
                            nc.scalar.activation(o_nat, pn, AF.Copy)
                        else:
                            nc.vector.tensor_copy(o_nat, pn)
                        nc.sync.dma_start(
                            out=out_d[t0 + ti].rearrange("(a p) d -> p a d", p=P),
                            in_=o_nat,
                        )

                hist: list = []
                for c in range(NCHUNKS):
                    a_str = s_block(c)
                    if len(hist) >= 1:
                        av_c, av_a = hist[-1][0], hist[-1][1]
                        o_sbs = av_block(av_c, av_a)
                        hist[-1] = (av_c, av_a, o_sbs)
                    if len(hist) >= 2:
                        out_block(hist[0][0], hist[0][2])
                        hist.pop(0)
                    hist.append((c, a_str, None))
                # Drain.
                av_c, av_a = hist[-1][0], hist[-1][1]
                o_sbs = av_block(av_c, av_a)
                hist[-1] = (av_c, av_a, o_sbs)
                for ent in hist:
                    out_block(ent[0], ent[2])

    nc.finalize()
    return nc


def prepare_in_maps(inputs):
    x = np.ascontiguousarray(np.asarray(inputs["x"], dtype=np.float32))
    w_q = np.asarray(inputs["W_Q"], dtype=np.float32)
    w_k = np.asarray(inputs["W_K"], dtype=np.float32)
    w_v = np.asarray(inputs["W_V"], dtype=np.float32)

    wqt = np.ascontiguousarray(w_q.T).astype(ml_dtypes.bfloat16)
    wkt = np.ascontiguousarray(w_k.T * np.float32(1.0 / np.sqrt(DH))).astype(
        ml_dtypes.bfloat16
    )
    wvt = np.ascontiguousarray(w_v.T).astype(ml_dtypes.bfloat16)

    return [
        {"x": np.ascontiguousarray(x[b]), "wqt": wqt, "wkt": wkt, "wvt": wvt}
        for b in range(B)
    ]


def kernel(**inputs) -> np.ndarray:
    if "nc" not in _CACHE:
        _CACHE["nc"] = _build_program()
    nc = _CACHE["nc"]

    in_maps = prepare_in_maps(inputs)
    res = run_bass_kernel_spmd(nc, in_maps, core_ids=list(range(B)))
    out = np.stack([res.results[b]["out"] for b in range(B)], axis=0)
    return out.reshape(B, T, N, D)


# revision 13
# speedup vs baseline: 1.6805x; 1.6805x over previous
"""Trainium2 Bass kernel for nn_MultiHeadCDGCN (v4).

Math (per batch b):
  t_w  = softmax(x, axis=T);  TAtt = sum_T(x * t_w)          [N, D]
  Q    = x @ W_Q.T                                           [T, N, D]
  K    = TAtt @ W_K.T ; V = TAtt @ W_V.T                     [N, D]
  S_th = Q_th @ K_h.T / sqrt(dh)   (per t, head h)           [N, N]
  out  = (relu(S) + I) @ V = relu(S) @ V + V                 [T, N, D]

Sharding: data-parallel over B across 8 NeuronCores (B == 8, one batch
per core); no collectives. The device computes out^T [D, T*N] per
batch; the host unshard step restores [T, N, D] layout.

Structure:
  - S / A@V / Q matmuls in bf16 (fp32 matmuls lower to 2 HW passes;
    bf16 is 1 pass at 1 col/cycle). Softmax stats stay fp32-accurate:
    exp reads the fp32 x^T PSUM directly.
  - GPSIMD cannot touch PSUM on TRN2, so every PSUM evacuation is on
    ACT/DVE; GPSIMD owns the SBUF-only sum_e accumulation; sum_xe
    accumulates on the PE as identity-matmul PSUM accumulation (one
    PSUM bank per accumulation group - sharing a bank corrupts it).
  - Phase C interleaves S and A@V instruction pairs with a 2-chunk
    software pipeline so the PE never idles (idle gaps trigger HAM
    re-throttle; HW then runs matmuls below full clock).
  - +V is fused into the po evacuation as scalar_tensor_tensor.
"""

import sys

import numpy as np

sys.path.insert(0, "/opt/trn_rl_repo")

import ml_dtypes  # noqa: E402

import concourse.bacc as bacc  # noqa: E402
import concourse.tile as tile  # noqa: E402
from concourse import mybir  # noqa: E402
from concourse.masks import make_identity  # noqa: E402
from concourse.bass_utils import run_bass_kernel_spmd  # noqa: E402

F32 = mybir.dt.float32
BF16 = mybir.dt.bfloat16
AF = mybir.ActivationFunctionType
ALU = mybir.AluOpType

B, T, N, D, H, DH = 8, 32, 256, 256, 8, 32
P = 128
NCHUNKS = 16  # tn chunks of 512 (2 frames each)
CHUNK_T = 2
CHUNK_TN = CHUNK_T * N  # 512

_CACHE: dict = {}


def _build_program():
    nc = bacc.Bacc()

    x_d = nc.dram_tensor("x", [T, N, D], F32, kind="ExternalInput")
    wqt_d = nc.dram_tensor("wqt", [D, D], BF16, kind="ExternalInput")
    wkt_d = nc.dram_tensor("wkt", [D, D], BF16, kind="ExternalInput")
    wvt_d = nc.dram_tensor("wvt", [D, D], BF16, kind="ExternalInput")
    out_d = nc.dram_tensor("out", [D, T * N], F32, kind="ExternalOutput")

    with tile.TileContext(nc) as tc:
        with (
            tc.tile_pool(name="consts", bufs=1) as consts,
            tc.tile_pool(name="xa", bufs=3) as xa_pool,
            tc.tile_pool(name="xt", bufs=3) as xt_pool,
            tc.tile_pool(name="ew", bufs=8) as e_pool,
            tc.tile_pool(name="at", bufs=24) as a_pool,
            tc.tile_pool(name="ot", bufs=4) as o_pool,
            tc.tile_pool(name="misc", bufs=1) as misc,
        ):
            eye = consts.tile([P, P], F32)
            make_identity(nc, eye)
            eye_bf = consts.tile([P, P], BF16)
            nc.vector.tensor_copy(eye_bf, eye)

            # Weights [k, j], k split over 2 partition tiles, bf16.
            wqt_sb = consts.tile([P, 2, D], BF16)
            wkt_sb = consts.tile([P, 2, D], BF16)
            wvt_sb = consts.tile([P, 2, D], BF16)
            for w_sb, w_d in ((wqt_sb, wqt_d), (wkt_sb, wkt_d), (wvt_sb, wvt_d)):
                for kc in range(2):
                    nc.sync.dma_start(
                        out=w_sb[:, kc, :],
                        in_=w_d[kc * P : (kc + 1) * P, :],
                    )

            # Q.T strip [j, tn] resident, bf16 (j split over 2 tiles).
            qt_sb = consts.tile([P, 2, T * N], BF16)

            # sum_e wide accumulator (SBUF, GPSIMD-owned).
            acc_e = consts.tile([P, 2, CHUNK_TN], F32)
            nc.gpsimd.memset(acc_e, 0.0)

            # ============ Phase A + B (stat PSUM pool scoped) ============
            with (
                tc.tile_pool(name="ps_t", bufs=3, space="PSUM") as ps_t,
                tc.tile_pool(name="ps_q", bufs=2, space="PSUM") as ps_q,
                tc.tile_pool(name="ps_s", bufs=1, space="PSUM") as ps_s,
            ):
                # sum_xe accumulators: one full PSUM bank per dc so the two
                # long-lived accumulation groups never share a bank.
                acc_xe_t = [
                    ps_s.tile([P, CHUNK_TN], F32, name=f"accxe{dc}")
                    for dc in range(2)
                ]
                acc_xe = {dc: acc_xe_t[dc][:, 0:N] for dc in range(2)}

                xe_strips = {}

                def stats_q_block(c, xt):
                    """xe stat-acc + Q projection for chunk c (PE work)."""
                    xe_t = xe_strips.pop(c)
                    for dc in range(2):
                        for ti in range(CHUNK_T):
                            first = c == 0 and ti == 0
                            last = c == NCHUNKS - 1 and ti == CHUNK_T - 1
                            nc.tensor.matmul(
                                acc_xe[dc],
                                eye_bf,
                                xe_t[:, dc, ti * N : (ti + 1) * N],
                                start=first,
                                stop=last,
                                skip_group_check=True,
                            )
                    for jc in range(2):
                        pq = ps_q.tile(
                            [P, CHUNK_TN], F32, tag="pq", name=f"pq{jc}"
                        )
                        for kc in range(2):
                            nc.tensor.matmul(
                                pq,
                                wqt_sb[:, kc, jc * P : (jc + 1) * P],
                                xt[:, kc, :],
                                start=(kc == 0),
                                stop=(kc == 1),
                            )
                        dst = qt_sb[:, jc, c * CHUNK_TN : (c + 1) * CHUNK_TN]
                        if jc == 0:
                            nc.scalar.activation(dst, pq, AF.Copy)
                        else:
                            nc.vector.tensor_copy(dst, pq)

                prev = None  # (c, xt)
                for c in range(NCHUNKS):
                    t0 = c * CHUNK_T
                    xa = xa_pool.tile([P, 4, D], F32)
                    nc.sync.dma_start(
                        out=xa,
                        in_=x_d[t0 : t0 + CHUNK_T].rearrange(
                            "t (s p) d -> p (t s) d", p=P
                        ),
                    )

                    xt = xt_pool.tile([P, 2, CHUNK_TN], BF16)
                    xe_t = e_pool.tile([P, 2, CHUNK_TN], BF16, name="xe")
                    xe_strips[c] = xe_t
                    for dc in range(2):
                        pt = ps_t.tile(
                            [P, CHUNK_TN], F32, tag="pt", name=f"pt{dc}"
                        )
                        for s in range(4):
                            nc.tensor.transpose(
                                pt[:, s * P : (s + 1) * P],
                                xa[:, s, dc * P : (dc + 1) * P],
                                eye,
                            )
                        nc.scalar.activation(xt[:, dc, :], pt, AF.Copy)
                        e_t = e_pool.tile([P, CHUNK_TN], BF16, name="e")
                        nc.scalar.activation(e_t, pt, AF.Exp)
                        nc.vector.tensor_mul(xe_t[:, dc, :], xt[:, dc, :], e_t)
                        nc.gpsimd.tensor_add(
                            acc_e[:, dc, :], acc_e[:, dc, :], e_t
                        )
                    # One-chunk software pipeline keeps the PE off the
                    # ACT/DVE critical path of the current chunk.
                    if prev is not None:
                        stats_q_block(*prev)
                    prev = (c, xt)
                stats_q_block(*prev)

                # ---------------- Phase B: TAtt.T, K.T, V.T, V
                sum_e = misc.tile([P, 2, N], F32)
                for dc in range(2):
                    nc.vector.tensor_add(
                        sum_e[:, dc, :],
                        acc_e[:, dc, 0:N],
                        acc_e[:, dc, N : 2 * N],
                    )
                rec = misc.tile([P, 2, N], F32)
                tatt_t = misc.tile([P, 2, N], BF16)  # TAtt.T [d, n] bf16
                for dc in range(2):
                    nc.vector.reciprocal(rec[:, dc, :], sum_e[:, dc, :])
                    nc.vector.scalar_tensor_tensor(
                        out=tatt_t[:, dc, :],
                        in0=acc_xe[dc],
                        scalar=1.0,
                        in1=rec[:, dc, :],
                        op0=ALU.mult,
                        op1=ALU.mult,
                    )

                kt_sb = consts.tile([P, 2, N], BF16)  # K.T [j, m] (pre-scaled)
                vt2 = consts.tile([P, 2, 2, N], F32)  # V.T doubled per hg
                for w_sb, is_v in ((wkt_sb, 0), (wvt_sb, 1)):
                    for jc in range(2):
                        pk = ps_q.tile([P, N], F32, tag="pq", name="pk")
                        for kc in range(2):
                            nc.tensor.matmul(
                                pk,
                                w_sb[:, kc, jc * P : (jc + 1) * P],
                                tatt_t[:, kc, :],
                                start=(kc == 0),
                                stop=(kc == 1),
                            )
                        if not is_v:
                            nc.vector.tensor_copy(kt_sb[:, jc, :], pk)
                        else:
                            for ti in range(2):
                                nc.vector.tensor_copy(vt2[:, jc, ti, :], pk)

                v_sb = consts.tile([P, 2, D], BF16)  # V [m, j]
                for mc in range(2):
                    pv = ps_q.tile([P, D], F32, tag="pq", name="pv")
                    for kc in range(2):
                        nc.tensor.matmul(
                            pv,
                            tatt_t[:, kc, mc * P : (mc + 1) * P],
                            wvt_sb[:, kc, :],
                            start=(kc == 0),
                            stop=(kc == 1),
                        )
                    nc.scalar.activation(v_sb[:, mc, :], pv, AF.Copy)

            # ============ Phase C: attention + output ============
            with (
                tc.tile_pool(name="ps_a", bufs=3, space="PSUM") as ps_a,
                tc.tile_pool(name="ps_o", bufs=2, space="PSUM") as ps_o,
            ):
                # 5 ACT : 3 DVE relu-evac split.
                relu_acts = (0, 2, 4, 6, 7)

                def s_pair(c, k, a_str):
                    """S matmuls + relu evac for head-pair k of chunk c."""
                    hg, mc, rp = k >> 2, (k >> 1) & 1, k & 1
                    ps2 = ps_a.tile(
                        [P, 2 * CHUNK_TN], F32, tag="psa", name=f"ps{k}"
                    )
                    for rh in range(2):
                        r = rp * 2 + rh
                        nc.tensor.matmul(
                            ps2[:, rh * CHUNK_TN : (rh + 1) * CHUNK_TN],
                            kt_sb[
                                r * 32 : (r + 1) * 32, hg, mc * P : (mc + 1) * P
                            ],
                            qt_sb[
                                r * 32 : (r + 1) * 32,
                                hg,
                                c * CHUNK_TN : (c + 1) * CHUNK_TN,
                            ],
                            start=True,
                            stop=True,
                            tile_position=(r * 32, 0),
                        )
                    a2 = a_pool.tile(
                        [P, 2 * CHUNK_TN], BF16, tag="at", name=f"a{k}"
                    )
                    if k in relu_acts:
                        nc.scalar.activation(a2, ps2, AF.Relu)
                    else:
                        nc.vector.tensor_scalar_max(a2, ps2, 0.0)
                    for rh in range(2):
                        a_str[(hg, rp * 2 + rh, mc)] = a2[
                            :, rh * CHUNK_TN : (rh + 1) * CHUNK_TN
                        ]

                def av_pair(c, k, a_str, pos):
                    """A@V matmul pair k (of 8) for chunk c."""
                    for i in range(2):
                        j = 2 * k + i
                        hg, mc, r = j >> 3, (j >> 2) & 1, j & 3
                        if r == 0 and mc == 0:
                            pos[hg] = ps_o.tile(
                                [P, CHUNK_TN], F32, tag="po", name=f"po{hg}"
                            )
                        h = hg * 4 + r
                        nc.tensor.matmul(
                            pos[hg][r * 32 : (r + 1) * 32, :],
                            v_sb[:, mc, h * 32 : (h + 1) * 32],
                            a_str[(hg, r, mc)],
                            start=(mc == 0),
                            stop=(mc == 1),
                            tile_position=(0, r * 32),
                            skip_group_check=True,
                        )

                def po_evac(c, pos):
                    """+V fused evacuation of po, then DMA of out^T strip."""
                    for hg in range(2):
                        o_sb = o_pool.tile([P, CHUNK_TN], F32, name=f"o{hg}")
                        nc.vector.scalar_tensor_tensor(
                            out=o_sb,
                            in0=pos[hg],
                            scalar=1.0,
                            in1=vt2[:, hg, :, :],
                            op0=ALU.mult,
                            op1=ALU.add,
                        )
                        nc.sync.dma_start(
                            out=out_d[
                                hg * P : (hg + 1) * P,
                                c * CHUNK_TN : (c + 1) * CHUNK_TN,
                            ],
                            in_=o_sb,
                        )

                astrs = {}
                poss = {}
                for c in range(NCHUNKS + 2):
                    if c < NCHUNKS:
                        astrs[c] = {}
                    if c - 2 >= 0:
                        poss[c - 2] = {}
                    for k in range(8):
                        if c < NCHUNKS:
                            s_pair(c, k, astrs[c])
                        if c - 2 >= 0:
                            av_pair(c - 2, k, astrs[c - 2], poss[c - 2])
                    if c - 2 >= 0:
                        po_evac(c - 2, poss.pop(c - 2))
                        astrs.pop(c - 2)

    nc.finalize()
    return nc


def prepare_in_maps(inputs):
    x = np.ascontiguousarray(np.asarray(inputs["x"], dtype=np.float32))
    w_q = np.asarray(inputs["W_Q"], dtype=np.float32)
    w_k = np.asarray(inputs["W_K"], dtype=np.float32)
    w_v = np.asarray(inputs["W_V"], dtype=np.float32)

    wqt = np.ascontiguousarray(w_q.T).astype(ml_dtypes.bfloat16)
    wkt = np.ascontiguousarray(w_k.T * np.float32(1.0 / np.sqrt(DH))).astype(
        ml_dtypes.bfloat16
    )
    wvt = np.ascontiguousarray(w_v.T).astype(ml_dtypes.bfloat16)

    return [
        {"x": np.ascontiguousarray(x[b]), "wqt": wqt, "wkt": wkt, "wvt": wvt}
        for b in range(B)
    ]


def kernel(**inputs) -> np.ndarray:
    if "nc" not in _CACHE:
        _CACHE["nc"] = _build_program()
    nc = _CACHE["nc"]

    in_maps = prepare_in_maps(inputs)
    res = run_bass_kernel_spmd(nc, in_maps, core_ids=list(range(B)))
    # Device emits out^T [D, T*N]; restore [T, N, D] during unshard.
    out = np.stack(
        [np.ascontiguousarray(res.results[b]["out"].T) for b in range(B)],
        axis=0,
    )
    return out.reshape(B, T, N, D)


# revision 16
# speedup vs baseline: 1.6808x; 1.0001x over previous
"""Trainium2 Bass kernel for nn_MultiHeadCDGCN (v4).

Math (per batch b):
  t_w  = softmax(x, axis=T);  TAtt = sum_T(x * t_w)          [N, D]
  Q    = x @ W_Q.T                                           [T, N, D]
  K    = TAtt @ W_K.T ; V = TAtt @ W_V.T                     [N, D]
  S_th = Q_th @ K_h.T / sqrt(dh)   (per t, head h)           [N, N]
  out  = (relu(S) + I) @ V = relu(S) @ V + V                 [T, N, D]

Sharding: data-parallel over B across 8 NeuronCores (B == 8, one batch
per core); no collectives. The device computes out^T [D, T*N] per
batch; the host unshard step restores [T, N, D] layout.

Structure:
  - S / A@V / Q matmuls in bf16 (fp32 matmuls lower to 2 HW passes;
    bf16 is 1 pass at 1 col/cycle). Softmax stats stay fp32-accurate:
    exp reads the fp32 x^T PSUM directly.
  - GPSIMD cannot touch PSUM on TRN2, so every PSUM evacuation is on
    ACT/DVE; GPSIMD owns the SBUF-only sum_e accumulation; sum_xe
    accumulates on the PE as identity-matmul PSUM accumulation (one
    PSUM bank per accumulation group - sharing a bank corrupts it).
  - Phase C interleaves S and A@V instruction pairs with a 2-chunk
    software pipeline so the PE never idles (idle gaps trigger HAM
    re-throttle; HW then runs matmuls below full clock).
  - +V is fused into the po evacuation as scalar_tensor_tensor.
"""

import sys

import numpy as np

sys.path.insert(0, "/opt/trn_rl_repo")

import ml_dtypes  # noqa: E402

import concourse.bacc as bacc  # noqa: E402
import concourse.tile as tile  # noqa: E402
from concourse import mybir  # noqa: E402
from concourse.masks import make_identity  # noqa: E402
from concourse.bass_utils import run_bass_kernel_spmd  # noqa: E402

F32 = mybir.dt.float32
BF16 = mybir.dt.bfloat16
AF = mybir.ActivationFunctionType
ALU = mybir.AluOpType

B, T, N, D, H, DH = 8, 32, 256, 256, 8, 32
P = 128
NCHUNKS = 16  # tn chunks of 512 (2 frames each)
CHUNK_T = 2
CHUNK_TN = CHUNK_T * N  # 512

_CACHE: dict = {}


def _build_program():
    nc = bacc.Bacc()

    x_d = nc.dram_tensor("x", [T, N, D], F32, kind="ExternalInput")
    wqt_d = nc.dram_tensor("wqt", [D, D], BF16, kind="ExternalInput")
    wkt_d = nc.dram_tensor("wkt", [D, D], BF16, kind="ExternalInput")
    wvt_d = nc.dram_tensor("wvt", [D, D], BF16, kind="ExternalInput")
    out_d = nc.dram_tensor("out", [D, T * N], F32, kind="ExternalOutput")

    with tile.TileContext(nc) as tc:
        with (
            tc.tile_pool(name="consts", bufs=1) as consts,
            tc.tile_pool(name="xa", bufs=3) as xa_pool,
            tc.tile_pool(name="xt", bufs=3) as xt_pool,
            tc.tile_pool(name="ew", bufs=8) as e_pool,
            tc.tile_pool(name="at", bufs=32) as a_pool,
            tc.tile_pool(name="ot", bufs=4) as o_pool,
            tc.tile_pool(name="misc", bufs=1) as misc,
        ):
            eye = consts.tile([P, P], F32)
            make_identity(nc, eye)
            eye_bf = consts.tile([P, P], BF16)
            nc.vector.tensor_copy(eye_bf, eye)

            # Weights [k, j], k split over 2 partition tiles, bf16.
            wqt_sb = consts.tile([P, 2, D], BF16)
            wkt_sb = consts.tile([P, 2, D], BF16)
            wvt_sb = consts.tile([P, 2, D], BF16)
            for w_sb, w_d in ((wqt_sb, wqt_d), (wkt_sb, wkt_d), (wvt_sb, wvt_d)):
                for kc in range(2):
                    nc.sync.dma_start(
                        out=w_sb[:, kc, :],
                        in_=w_d[kc * P : (kc + 1) * P, :],
                    )

            # Q.T strip [j, tn] resident, bf16 (j split over 2 tiles).
            qt_sb = consts.tile([P, 2, T * N], BF16)

            # sum_e wide accumulator (SBUF, GPSIMD-owned).
            acc_e = consts.tile([P, 2, CHUNK_TN], F32)
            nc.gpsimd.memset(acc_e, 0.0)

            # ============ Phase A + B (stat PSUM pool scoped) ============
            with (
                tc.tile_pool(name="ps_t", bufs=3, space="PSUM") as ps_t,
                tc.tile_pool(name="ps_q", bufs=2, space="PSUM") as ps_q,
                tc.tile_pool(name="ps_s", bufs=1, space="PSUM") as ps_s,
            ):
                # sum_xe accumulators: one full PSUM bank per dc so the two
                # long-lived accumulation groups never share a bank.
                acc_xe_t = [
                    ps_s.tile([P, CHUNK_TN], F32, name=f"accxe{dc}")
                    for dc in range(2)
                ]
                acc_xe = {dc: acc_xe_t[dc][:, 0:N] for dc in range(2)}

                xe_strips = {}

                def stats_q_block(c, xt):
                    """xe stat-acc + Q projection for chunk c (PE work)."""
                    xe_t = xe_strips.pop(c)
                    for dc in range(2):
                        for ti in range(CHUNK_T):
                            first = c == 0 and ti == 0
                            last = c == NCHUNKS - 1 and ti == CHUNK_T - 1
                            nc.tensor.matmul(
                                acc_xe[dc],
                                eye_bf,
                                xe_t[:, dc, ti * N : (ti + 1) * N],
                                start=first,
                                stop=last,
                                skip_group_check=True,
                            )
                    for jc in range(2):
                        pq = ps_q.tile(
                            [P, CHUNK_TN], F32, tag="pq", name=f"pq{jc}"
                        )
                        for kc in range(2):
                            nc.tensor.matmul(
                                pq,
                                wqt_sb[:, kc, jc * P : (jc + 1) * P],
                                xt[:, kc, :],
                                start=(kc == 0),
                                stop=(kc == 1),
                            )
                        dst = qt_sb[:, jc, c * CHUNK_TN : (c + 1) * CHUNK_TN]
                        if jc == 0:
                            nc.scalar.activation(dst, pq, AF.Copy)
                        else:
                            nc.vector.tensor_copy(dst, pq)

                pipe = []  # [(c, xt), ...]
                for c in range(NCHUNKS):
                    t0 = c * CHUNK_T
                    xa = xa_pool.tile([P, 4, D], F32)
                    nc.sync.dma_start(
                        out=xa,
                        in_=x_d[t0 : t0 + CHUNK_T].rearrange(
                            "t (s p) d -> p (t s) d", p=P
                        ),
                    )

                    xt = xt_pool.tile([P, 2, CHUNK_TN], BF16)
                    xe_t = e_pool.tile([P, 2, CHUNK_TN], BF16, name="xe")
                    xe_strips[c] = xe_t
                    for dc in range(2):
                        pt = ps_t.tile(
                            [P, CHUNK_TN], F32, tag="pt", name=f"pt{dc}"
                        )
                        for s in range(4):
                            nc.tensor.transpose(
                                pt[:, s * P : (s + 1) * P],
                                xa[:, s, dc * P : (dc + 1) * P],
                                eye,
                            )
                        nc.scalar.activation(xt[:, dc, :], pt, AF.Copy)
                        e_t = e_pool.tile([P, CHUNK_TN], BF16, name="e")
                        nc.scalar.activation(e_t, pt, AF.Exp)
                        nc.vector.tensor_mul(xe_t[:, dc, :], xt[:, dc, :], e_t)
                        nc.gpsimd.tensor_add(
                            acc_e[:, dc, :], acc_e[:, dc, :], e_t
                        )
                    # Two-chunk software pipeline keeps the PE off the
                    # ACT/DVE critical path of recent chunks.
                    pipe.append((c, xt))
                    if len(pipe) > 2:
                        stats_q_block(*pipe.pop(0))
                for ent in pipe:
                    stats_q_block(*ent)

                # ---------------- Phase B: TAtt.T, K.T, V.T, V
                sum_e = misc.tile([P, 2, N], F32)
                for dc in range(2):
                    nc.vector.tensor_add(
                        sum_e[:, dc, :],
                        acc_e[:, dc, 0:N],
                        acc_e[:, dc, N : 2 * N],
                    )
                rec = misc.tile([P, 2, N], F32)
                tatt_t = misc.tile([P, 2, N], BF16)  # TAtt.T [d, n] bf16
                for dc in range(2):
                    nc.vector.reciprocal(rec[:, dc, :], sum_e[:, dc, :])
                    nc.vector.scalar_tensor_tensor(
                        out=tatt_t[:, dc, :],
                        in0=acc_xe[dc],
                        scalar=1.0,
                        in1=rec[:, dc, :],
                        op0=ALU.mult,
                        op1=ALU.mult,
                    )

                kt_sb = consts.tile([P, 2, N], BF16)  # K.T [j, m] (pre-scaled)
                vt2 = consts.tile([P, 2, 2, N], F32)  # V.T doubled per hg
                for w_sb, is_v in ((wkt_sb, 0), (wvt_sb, 1)):
                    for jc in range(2):
                        pk = ps_q.tile([P, N], F32, tag="pq", name="pk")
                        for kc in range(2):
                            nc.tensor.matmul(
                                pk,
                                w_sb[:, kc, jc * P : (jc + 1) * P],
                                tatt_t[:, kc, :],
                                start=(kc == 0),
                                stop=(kc == 1),
                            )
                        if not is_v:
                            nc.vector.tensor_copy(kt_sb[:, jc, :], pk)
                        else:
                            for ti in range(2):
                                nc.vector.tensor_copy(vt2[:, jc, ti, :], pk)

                v_sb = consts.tile([P, 2, D], BF16)  # V [m, j]
                for mc in range(2):
                    pv = ps_q.tile([P, D], F32, tag="pq", name="pv")
                    for kc in range(2):
                        nc.tensor.matmul(
                            pv,
                            tatt_t[:, kc, mc * P : (mc + 1) * P],
                            wvt_sb[:, kc, :],
                            start=(kc == 0),
                            stop=(kc == 1),
                        )
                    nc.scalar.activation(v_sb[:, mc, :], pv, AF.Copy)

            # ============ Phase C: attention + output ============
            with (
                tc.tile_pool(name="ps_a", bufs=3, space="PSUM") as ps_a,
                tc.tile_pool(name="ps_o", bufs=2, space="PSUM") as ps_o,
            ):
                # 5 ACT : 3 DVE relu-evac split.
                relu_acts = (0, 2, 4, 6, 7)

                def s_pair(c, k, a_str):
                    """S matmuls + relu evac for head-pair k of chunk c."""
                    hg, mc, rp = k >> 2, (k >> 1) & 1, k & 1
                    ps2 = ps_a.tile(
                        [P, 2 * CHUNK_TN], F32, tag="psa", name=f"ps{k}"
                    )
                    for rh in range(2):
                        r = rp * 2 + rh
                        nc.tensor.matmul(
                            ps2[:, rh * CHUNK_TN : (rh + 1) * CHUNK_TN],
                            kt_sb[
                                r * 32 : (r + 1) * 32, hg, mc * P : (mc + 1) * P
                            ],
                            qt_sb[
                                r * 32 : (r + 1) * 32,
                                hg,
                                c * CHUNK_TN : (c + 1) * CHUNK_TN,
                            ],
                            start=True,
                            stop=True,
                            tile_position=(r * 32, 0),
                        )
                    a2 = a_pool.tile(
                        [P, 2 * CHUNK_TN], BF16, tag="at", name=f"a{k}"
                    )
                    if k in relu_acts:
                        nc.scalar.activation(a2, ps2, AF.Relu)
                    else:
                        nc.vector.tensor_scalar_max(a2, ps2, 0.0)
                    for rh in range(2):
                        a_str[(hg, rp * 2 + rh, mc)] = a2[
                            :, rh * CHUNK_TN : (rh + 1) * CHUNK_TN
                        ]

                def av_pair(c, k, a_str, pos):
                    """A@V matmul pair k (of 8) for chunk c."""
                    for i in range(2):
                        j = 2 * k + i
                        hg, mc, r = j >> 3, (j >> 2) & 1, j & 3
                        if r == 0 and mc == 0:
                            pos[hg] = ps_o.tile(
                                [P, CHUNK_TN], F32, tag="po", name=f"po{hg}"
                            )
                        h = hg * 4 + r
                        nc.tensor.matmul(
                            pos[hg][r * 32 : (r + 1) * 32, :],
                            v_sb[:, mc, h * 32 : (h + 1) * 32],
                            a_str[(hg, r, mc)],
                            start=(mc == 0),
                            stop=(mc == 1),
                            tile_position=(0, r * 32),
                            skip_group_check=True,
                        )

                def po_evac(c, pos):
                    """+V fused evacuation of po, then DMA of out^T strip."""
                    for hg in range(2):
                        o_sb = o_pool.tile([P, CHUNK_TN], F32, name=f"o{hg}")
                        nc.vector.scalar_tensor_tensor(
                            out=o_sb,
                            in0=pos[hg],
                            scalar=1.0,
                            in1=vt2[:, hg, :, :],
                            op0=ALU.mult,
                            op1=ALU.add,
                        )
                        nc.sync.dma_start(
                            out=out_d[
                                hg * P : (hg + 1) * P,
                                c * CHUNK_TN : (c + 1) * CHUNK_TN,
                            ],
                            in_=o_sb,
                        )

                astrs = {}
                poss = {}
                for c in range(NCHUNKS + 2):
                    if c < NCHUNKS:
                        astrs[c] = {}
                    if c - 2 >= 0:
                        poss[c - 2] = {}
                    for k in range(8):
                        if c < NCHUNKS:
                            s_pair(c, k, astrs[c])
                        if c - 2 >= 0:
                            av_pair(c - 2, k, astrs[c - 2], poss[c - 2])
                    if c - 2 >= 0:
                        po_evac(c - 2, poss.pop(c - 2))
                        astrs.pop(c - 2)

    nc.finalize()
    return nc


def prepare_in_maps(inputs):
    x = np.ascontiguousarray(np.asarray(inputs["x"], dtype=np.float32))
    w_q = np.asarray(inputs["W_Q"], dtype=np.float32)
    w_k = np.asarray(inputs["W_K"], dtype=np.float32)
    w_v = np.asarray(inputs["W_V"], dtype=np.float32)

    wqt = np.ascontiguousarray(w_q.T).astype(ml_dtypes.bfloat16)
    wkt = np.ascontiguousarray(w_k.T * np.float32(1.0 / np.sqrt(DH))).astype(
        ml_dtypes.bfloat16
    )
    wvt = np.ascontiguousarray(w_v.T).astype(ml_dtypes.bfloat16)

    return [
        {"x": np.ascontiguousarray(x[b]), "wqt": wqt, "wkt": wkt, "wvt": wvt}
        for b in range(B)
    ]


def kernel(**inputs) -> np.ndarray:
    if "nc" not in _CACHE:
        _CACHE["nc"] = _build_program()
    nc = _CACHE["nc"]

    in_maps = prepare_in_maps(inputs)
    res = run_bass_kernel_spmd(nc, in_maps, core_ids=list(range(B)))
    # Device emits out^T [D, T*N]; restore [T, N, D] during unshard.
    out = np.stack(
        [np.ascontiguousarray(res.results[b]["out"].T) for b in range(B)],
        axis=0,
    )
    return out.reshape(B, T, N, D)


# revision 18
# speedup vs baseline: 1.9965x; 1.1879x over previous
"""Trainium2 Bass kernel for nn_MultiHeadCDGCN (v4).

Math (per batch b):
  t_w  = softmax(x, axis=T);  TAtt = sum_T(x * t_w)          [N, D]
  Q    = x @ W_Q.T                                           [T, N, D]
  K    = TAtt @ W_K.T ; V = TAtt @ W_V.T                     [N, D]
  S_th = Q_th @ K_h.T / sqrt(dh)   (per t, head h)           [N, N]
  out  = (relu(S) + I) @ V = relu(S) @ V + V                 [T, N, D]

Sharding: data-parallel over B across 8 NeuronCores (B == 8, one batch
per core); no collectives. The device computes out^T [D, T*N] per
batch; the host unshard step restores [T, N, D] layout.

Structure:
  - S / A@V / Q matmuls in bf16 (fp32 matmuls lower to 2 HW passes;
    bf16 is 1 pass at 1 col/cycle). Softmax stats stay fp32-accurate:
    exp reads the fp32 x^T PSUM directly.
  - GPSIMD cannot touch PSUM on TRN2, so every PSUM evacuation is on
    ACT/DVE; GPSIMD owns the SBUF-only sum_e accumulation; sum_xe
    accumulates on the PE as identity-matmul PSUM accumulation (one
    PSUM bank per accumulation group - sharing a bank corrupts it).
  - Phase C interleaves S and A@V instruction pairs with a 2-chunk
    software pipeline so the PE never idles (idle gaps trigger HAM
    re-throttle; HW then runs matmuls below full clock).
  - +V is fused into the po evacuation as scalar_tensor_tensor.
"""

import sys

import numpy as np

sys.path.insert(0, "/opt/trn_rl_repo")

import ml_dtypes  # noqa: E402

import concourse.bacc as bacc  # noqa: E402
import concourse.tile as tile  # noqa: E402
from concourse import mybir  # noqa: E402
from concourse.masks import make_identity  # noqa: E402
from concourse.bass_utils import run_bass_kernel_spmd  # noqa: E402

F32 = mybir.dt.float32
BF16 = mybir.dt.bfloat16
AF = mybir.ActivationFunctionType
ALU = mybir.AluOpType

B, T, N, D, H, DH = 8, 32, 256, 256, 8, 32
P = 128
NCHUNKS = 16  # tn chunks of 512 (2 frames each)
CHUNK_T = 2
CHUNK_TN = CHUNK_T * N  # 512

_CACHE: dict = {}


def _build_program():
    nc = bacc.Bacc()

    x_d = nc.dram_tensor("x", [T, N, D], F32, kind="ExternalInput")
    wqt_d = nc.dram_tensor("wqt", [D, D], BF16, kind="ExternalInput")
    wkt_d = nc.dram_tensor("wkt", [D, D], BF16, kind="ExternalInput")
    wvt_d = nc.dram_tensor("wvt", [D, D], BF16, kind="ExternalInput")
    out_d = nc.dram_tensor("out", [D, T * N], F32, kind="ExternalOutput")

    with tile.TileContext(nc) as tc:
        with (
            tc.tile_pool(name="consts", bufs=1) as consts,
            tc.tile_pool(name="xa", bufs=3) as xa_pool,
            tc.tile_pool(name="xt", bufs=3) as xt_pool,
            tc.tile_pool(name="ew", bufs=8) as e_pool,
            tc.tile_pool(name="at", bufs=32) as a_pool,
            tc.tile_pool(name="ot", bufs=4) as o_pool,
            tc.tile_pool(name="misc", bufs=1) as misc,
        ):
            eye = consts.tile([P, P], F32)
            make_identity(nc, eye)
            eye_bf = consts.tile([P, P], BF16)
            nc.vector.tensor_copy(eye_bf, eye)

            # Weights [k, j], k split over 2 partition tiles, bf16.
            wqt_sb = consts.tile([P, 2, D], BF16)
            wkt_sb = consts.tile([P, 2, D], BF16)
            wvt_sb = consts.tile([P, 2, D], BF16)
            for w_sb, w_d in ((wqt_sb, wqt_d), (wkt_sb, wkt_d), (wvt_sb, wvt_d)):
                for kc in range(2):
                    nc.sync.dma_start(
                        out=w_sb[:, kc, :],
                        in_=w_d[kc * P : (kc + 1) * P, :],
                    )

            # Q.T strip [j, tn] resident, bf16 (j split over 2 tiles).
            qt_sb = consts.tile([P, 2, T * N], BF16)

            # sum_e wide accumulator (SBUF, GPSIMD-owned).
            acc_e = consts.tile([P, 2, CHUNK_TN], F32)
            nc.gpsimd.memset(acc_e, 0.0)

            # ============ Phase A + B (stat PSUM pool scoped) ============
            with (
                tc.tile_pool(name="ps_t", bufs=3, space="PSUM") as ps_t,
                tc.tile_pool(name="ps_q", bufs=2, space="PSUM") as ps_q,
                tc.tile_pool(name="ps_s", bufs=1, space="PSUM") as ps_s,
            ):
                # sum_xe accumulators: one full PSUM bank per dc so the two
                # long-lived accumulation groups never share a bank.
                acc_xe_t = [
                    ps_s.tile([P, CHUNK_TN], F32, name=f"accxe{dc}")
                    for dc in range(2)
                ]
                acc_xe = {dc: acc_xe_t[dc] for dc in range(2)}

                xe_strips = {}

                def stats_q_block(c, xt):
                    """xe stat-acc + Q projection for chunk c (PE work)."""
                    xe_t = xe_strips.pop(c)
                    for dc in range(2):
                        nc.tensor.matmul(
                            acc_xe[dc],
                            eye_bf,
                            xe_t[:, dc, :],
                            start=(c == 0),
                            stop=(c == NCHUNKS - 1),
                            skip_group_check=True,
                        )
                    for jc in range(2):
                        pq = ps_q.tile(
                            [P, CHUNK_TN], F32, tag="pq", name=f"pq{jc}"
                        )
                        for kc in range(2):
                            nc.tensor.matmul(
                                pq,
                                wqt_sb[:, kc, jc * P : (jc + 1) * P],
                                xt[:, kc, :],
                                start=(kc == 0),
                                stop=(kc == 1),
                            )
                        dst = qt_sb[:, jc, c * CHUNK_TN : (c + 1) * CHUNK_TN]
                        nc.vector.tensor_copy(dst, pq)

                pipe = []  # [(c, xt), ...]
                for c in range(NCHUNKS):
                    t0 = c * CHUNK_T
                    xa = xa_pool.tile([P, 4, D], F32)
                    nc.sync.dma_start(
                        out=xa,
                        in_=x_d[t0 : t0 + CHUNK_T].rearrange(
                            "t (s p) d -> p (t s) d", p=P
                        ),
                    )

                    xt = xt_pool.tile([P, 2, CHUNK_TN], BF16)
                    xe_t = e_pool.tile([P, 2, CHUNK_TN], BF16, name="xe")
                    xe_strips[c] = xe_t
                    for dc in range(2):
                        pt = ps_t.tile(
                            [P, CHUNK_TN], F32, tag="pt", name=f"pt{dc}"
                        )
                        for s in range(4):
                            nc.tensor.transpose(
                                pt[:, s * P : (s + 1) * P],
                                xa[:, s, dc * P : (dc + 1) * P],
                                eye,
                            )
                        nc.scalar.activation(xt[:, dc, :], pt, AF.Copy)
                        e_t = e_pool.tile([P, CHUNK_TN], BF16, name="e")
                        nc.scalar.activation(e_t, pt, AF.Exp)
                        nc.vector.tensor_mul(xe_t[:, dc, :], pt, e_t)
                        nc.gpsimd.tensor_add(
                            acc_e[:, dc, :], acc_e[:, dc, :], e_t
                        )
                    # Two-chunk software pipeline keeps the PE off the
                    # ACT/DVE critical path of recent chunks.
                    pipe.append((c, xt))
                    if len(pipe) > 2:
                        stats_q_block(*pipe.pop(0))
                for ent in pipe:
                    stats_q_block(*ent)

                # ---------------- Phase B: TAtt.T, K.T, V.T, V
                sum_e = misc.tile([P, 2, N], F32)
                for dc in range(2):
                    nc.vector.tensor_add(
                        sum_e[:, dc, :],
                        acc_e[:, dc, 0:N],
                        acc_e[:, dc, N : 2 * N],
                    )
                rec = misc.tile([P, 2, N], F32)
                sxe_sb = misc.tile([P, 2, CHUNK_TN], F32)
                sum_xe = misc.tile([P, 2, N], F32)
                tatt_t = misc.tile([P, 2, N], BF16)  # TAtt.T [d, n] bf16
                for dc in range(2):
                    nc.vector.tensor_copy(sxe_sb[:, dc, :], acc_xe[dc])
                    nc.vector.tensor_add(
                        sum_xe[:, dc, :],
                        sxe_sb[:, dc, 0:N],
                        sxe_sb[:, dc, N : 2 * N],
                    )
                    nc.vector.reciprocal(rec[:, dc, :], sum_e[:, dc, :])
                    nc.vector.scalar_tensor_tensor(
                        out=tatt_t[:, dc, :],
                        in0=sum_xe[:, dc, :],
                        scalar=1.0,
                        in1=rec[:, dc, :],
                        op0=ALU.mult,
                        op1=ALU.mult,
                    )

                kt_sb = consts.tile([P, 2, N], BF16)  # K.T [j, m] (pre-scaled)
                vt2 = consts.tile([P, 2, 2, N], F32)  # V.T doubled per hg
                for w_sb, is_v in ((wkt_sb, 0), (wvt_sb, 1)):
                    for jc in range(2):
                        pk = ps_q.tile([P, N], F32, tag="pq", name="pk")
                        for kc in range(2):
                            nc.tensor.matmul(
                                pk,
                                w_sb[:, kc, jc * P : (jc + 1) * P],
                                tatt_t[:, kc, :],
                                start=(kc == 0),
                                stop=(kc == 1),
                            )
                        if not is_v:
                            nc.vector.tensor_copy(kt_sb[:, jc, :], pk)
                        else:
                            for ti in range(2):
                                nc.vector.tensor_copy(vt2[:, jc, ti, :], pk)

                v_sb = consts.tile([P, 2, D], BF16)  # V [m, j]
                for mc in range(2):
                    pv = ps_q.tile([P, D], F32, tag="pq", name="pv")
                    for kc in range(2):
                        nc.tensor.matmul(
                            pv,
                            tatt_t[:, kc, mc * P : (mc + 1) * P],
                            wvt_sb[:, kc, :],
                            start=(kc == 0),
                            stop=(kc == 1),
                        )
                    nc.scalar.activation(v_sb[:, mc, :], pv, AF.Copy)

            # ============ Phase C: attention + output ============
            with (
                tc.tile_pool(name="ps_a", bufs=3, space="PSUM") as ps_a,
                tc.tile_pool(name="ps_o", bufs=2, space="PSUM") as ps_o,
            ):
                # 5 ACT : 3 DVE relu-evac split.
                relu_acts = (0, 2, 4, 6, 7)

                def s_pair(c, k, a_str):
                    """S matmuls + relu evac for head-pair k of chunk c."""
                    hg, mc, rp = k >> 2, (k >> 1) & 1, k & 1
                    ps2 = ps_a.tile(
                        [P, 2 * CHUNK_TN], F32, tag="psa", name=f"ps{k}"
                    )
                    for rh in range(2):
                        r = rp * 2 + rh
                        nc.tensor.matmul(
                            ps2[:, rh * CHUNK_TN : (rh + 1) * CHUNK_TN],
                            kt_sb[
                                r * 32 : (r + 1) * 32, hg, mc * P : (mc + 1) * P
                            ],
                            qt_sb[
                                r * 32 : (r + 1) * 32,
                                hg,
                                c * CHUNK_TN : (c + 1) * CHUNK_TN,
                            ],
                            start=True,
                            stop=True,
                            tile_position=(r * 32, 0),
                        )
                    a2 = a_pool.tile(
                        [P, 2 * CHUNK_TN], BF16, tag="at", name=f"a{k}"
                    )
                    if k in relu_acts:
                        nc.scalar.activation(a2, ps2, AF.Relu)
                    else:
                        nc.vector.tensor_scalar_max(a2, ps2, 0.0)
                    for rh in range(2):
                        a_str[(hg, rp * 2 + rh, mc)] = a2[
                            :, rh * CHUNK_TN : (rh + 1) * CHUNK_TN
                        ]

                def av_pair(c, k, a_str, pos):
                    """A@V matmul pair k (of 8) for chunk c."""
                    for i in range(2):
                        j = 2 * k + i
                        hg, mc, r = j >> 3, (j >> 2) & 1, j & 3
                        if r == 0 and mc == 0:
                            pos[hg] = ps_o.tile(
                                [P, CHUNK_TN], F32, tag="po", name=f"po{hg}"
                            )
                        h = hg * 4 + r
                        nc.tensor.matmul(
                            pos[hg][r * 32 : (r + 1) * 32, :],
                            v_sb[:, mc, h * 32 : (h + 1) * 32],
                            a_str[(hg, r, mc)],
                            start=(mc == 0),
                            stop=(mc == 1),
                            tile_position=(0, r * 32),
                            skip_group_check=True,
                        )

                def po_evac(c, pos):
                    """+V fused evacuation of po, then DMA of out^T strip."""
                    for hg in range(2):
                        o_sb = o_pool.tile([P, CHUNK_TN], F32, name=f"o{hg}")
                        nc.vector.scalar_tensor_tensor(
                            out=o_sb,
                            in0=pos[hg],
                            scalar=1.0,
                            in1=vt2[:, hg, :, :],
                            op0=ALU.mult,
                            op1=ALU.add,
                        )
                        nc.sync.dma_start(
                            out=out_d[
                                hg * P : (hg + 1) * P,
                                c * CHUNK_TN : (c + 1) * CHUNK_TN,
                            ],
                            in_=o_sb,
                        )

                astrs = {}
                poss = {}
                for c in range(NCHUNKS + 2):
                    if c < NCHUNKS:
                        astrs[c] = {}
                    if c - 2 >= 0:
                        poss[c - 2] = {}
                    for kk in range(4):
                        for k in (2 * kk, 2 * kk + 1):
                            if c < NCHUNKS:
                                s_pair(c, k, astrs[c])
                        for k in (2 * kk, 2 * kk + 1):
                            if c - 2 >= 0:
                                av_pair(c - 2, k, astrs[c - 2], poss[c - 2])
                    if c - 2 >= 0:
                        po_evac(c - 2, poss.pop(c - 2))
                        astrs.pop(c - 2)

    nc.finalize()
    return nc


def prepare_in_maps(inputs):
    x = np.ascontiguousarray(np.asarray(inputs["x"], dtype=np.float32))
    w_q = np.asarray(inputs["W_Q"], dtype=np.float32)
    w_k = np.asarray(inputs["W_K"], dtype=np.float32)
    w_v = np.asarray(inputs["W_V"], dtype=np.float32)

    wqt = np.ascontiguousarray(w_q.T).astype(ml_dtypes.bfloat16)
    wkt = np.ascontiguousarray(w_k.T * np.float32(1.0 / np.sqrt(DH))).astype(
        ml_dtypes.bfloat16
    )
    wvt = np.ascontiguousarray(w_v.T).astype(ml_dtypes.bfloat16)

    return [
        {"x": np.ascontiguousarray(x[b]), "wqt": wqt, "wkt": wkt, "wvt": wvt}
        for b in range(B)
    ]


def kernel(**inputs) -> np.ndarray:
    if "nc" not in _CACHE:
        _CACHE["nc"] = _build_program()
    nc = _CACHE["nc"]

    in_maps = prepare_in_maps(inputs)
    res = run_bass_kernel_spmd(nc, in_maps, core_ids=list(range(B)))
    # Device emits out^T [D, T*N]; restore [T, N, D] during unshard.
    out = np.stack(
        [np.ascontiguousarray(res.results[b]["out"].T) for b in range(B)],
        axis=0,
    )
    return out.reshape(B, T, N, D)


# revision 20
# speedup vs baseline: 2.0184x; 1.0110x over previous
"""Trainium2 Bass kernel for nn_MultiHeadCDGCN (v4).

Math (per batch b):
  t_w  = softmax(x, axis=T);  TAtt = sum_T(x * t_w)          [N, D]
  Q    = x @ W_Q.T                                           [T, N, D]
  K    = TAtt @ W_K.T ; V = TAtt @ W_V.T                     [N, D]
  S_th = Q_th @ K_h.T / sqrt(dh)   (per t, head h)           [N, N]
  out  = (relu(S) + I) @ V = relu(S) @ V + V                 [T, N, D]

Sharding: data-parallel over B across 8 NeuronCores (B == 8, one batch
per core); no collectives. The device computes out^T [D, T*N] per
batch; the host unshard step restores [T, N, D] layout.

Structure:
  - S / A@V / Q matmuls in bf16 (fp32 matmuls lower to 2 HW passes;
    bf16 is 1 pass at 1 col/cycle). Softmax stats stay fp32-accurate:
    exp reads the fp32 x^T PSUM directly.
  - GPSIMD cannot touch PSUM on TRN2, so every PSUM evacuation is on
    ACT/DVE; GPSIMD owns the SBUF-only sum_e accumulation; sum_xe
    accumulates on the PE as identity-matmul PSUM accumulation (one
    PSUM bank per accumulation group - sharing a bank corrupts it).
  - Phase C interleaves S and A@V instruction pairs with a 2-chunk
    software pipeline so the PE never idles (idle gaps trigger HAM
    re-throttle; HW then runs matmuls below full clock).
  - +V is fused into the po evacuation as scalar_tensor_tensor.
"""

import sys

import numpy as np

sys.path.insert(0, "/opt/trn_rl_repo")

import ml_dtypes  # noqa: E402

import concourse.bacc as bacc  # noqa: E402
import concourse.tile as tile  # noqa: E402
from concourse import mybir  # noqa: E402
from concourse.masks import make_identity  # noqa: E402
from concourse.bass_utils import run_bass_kernel_spmd  # noqa: E402

F32 = mybir.dt.float32
BF16 = mybir.dt.bfloat16
AF = mybir.ActivationFunctionType
ALU = mybir.AluOpType

B, T, N, D, H, DH = 8, 32, 256, 256, 8, 32
P = 128
NCHUNKS = 16  # tn chunks of 512 (2 frames each)
CHUNK_T = 2
CHUNK_TN = CHUNK_T * N  # 512

_CACHE: dict = {}


def _build_program():
    nc = bacc.Bacc()

    x_d = nc.dram_tensor("x", [T, N, D], F32, kind="ExternalInput")
    wqt_d = nc.dram_tensor("wqt", [D, D], BF16, kind="ExternalInput")
    wkt_d = nc.dram_tensor("wkt", [D, D], BF16, kind="ExternalInput")
    wvt_d = nc.dram_tensor("wvt", [D, D], BF16, kind="ExternalInput")
    out_d = nc.dram_tensor("out", [D, T * N], F32, kind="ExternalOutput")

    with tile.TileContext(nc) as tc:
        with (
            tc.tile_pool(name="consts", bufs=1) as consts,
            tc.tile_pool(name="xa", bufs=3) as xa_pool,
            tc.tile_pool(name="xt", bufs=3) as xt_pool,
            tc.tile_pool(name="ew", bufs=8) as e_pool,
            tc.tile_pool(name="at", bufs=32) as a_pool,
            tc.tile_pool(name="ot", bufs=4) as o_pool,
            tc.tile_pool(name="misc", bufs=1) as misc,
        ):
            eye = consts.tile([P, P], F32)
            make_identity(nc, eye)
            eye_bf = consts.tile([P, P], BF16)
            nc.vector.tensor_copy(eye_bf, eye)

            # Weights [k, j], k split over 2 partition tiles, bf16.
            wqt_sb = consts.tile([P, 2, D], BF16)
            wkt_sb = consts.tile([P, 2, D], BF16)
            wvt_sb = consts.tile([P, 2, D], BF16)
            for w_sb, w_d in ((wqt_sb, wqt_d), (wkt_sb, wkt_d), (wvt_sb, wvt_d)):
                for kc in range(2):
                    nc.sync.dma_start(
                        out=w_sb[:, kc, :],
                        in_=w_d[kc * P : (kc + 1) * P, :],
                    )

            # Q.T strip [j, tn] resident, bf16 (j split over 2 tiles).
            qt_sb = consts.tile([P, 2, T * N], BF16)

            # sum_e wide accumulator (SBUF, GPSIMD-owned).
            acc_e = consts.tile([P, 2, CHUNK_TN], F32)
            nc.gpsimd.memset(acc_e, 0.0)

            # ============ Phase A + B (stat PSUM pool scoped) ============
            with (
                tc.tile_pool(name="ps_t", bufs=3, space="PSUM") as ps_t,
                tc.tile_pool(name="ps_q", bufs=3, space="PSUM") as ps_q,
                tc.tile_pool(name="ps_s", bufs=1, space="PSUM") as ps_s,
            ):
                # sum_xe accumulators: one full PSUM bank per dc so the two
                # long-lived accumulation groups never share a bank.
                acc_xe_t = [
                    ps_s.tile([P, CHUNK_TN], F32, name=f"accxe{dc}")
                    for dc in range(2)
                ]
                acc_xe = {dc: acc_xe_t[dc] for dc in range(2)}

                xe_strips = {}

                def stats_q_block(c, xt):
                    """xe stat-acc + Q projection for chunk c (PE work)."""
                    xe_t = xe_strips.pop(c)
                    for dc in range(2):
                        nc.tensor.matmul(
                            acc_xe[dc],
                            eye_bf,
                            xe_t[:, dc, :],
                            start=(c == 0),
                            stop=(c == NCHUNKS - 1),
                            skip_group_check=True,
                        )
                    for jc in range(2):
                        pq = ps_q.tile(
                            [P, CHUNK_TN], F32, tag="pq", name=f"pq{jc}"
                        )
                        for kc in range(2):
                            nc.tensor.matmul(
                                pq,
                                wqt_sb[:, kc, jc * P : (jc + 1) * P],
                                xt[:, kc, :],
                                start=(kc == 0),
                                stop=(kc == 1),
                            )
                        dst = qt_sb[:, jc, c * CHUNK_TN : (c + 1) * CHUNK_TN]
                        nc.vector.tensor_copy(dst, pq)

                pipe = []  # [(c, xt), ...]
                for c in range(NCHUNKS):
                    t0 = c * CHUNK_T
                    xa = xa_pool.tile([P, 4, D], F32)
                    nc.sync.dma_start(
                        out=xa,
                        in_=x_d[t0 : t0 + CHUNK_T].rearrange(
                            "t (s p) d -> p (t s) d", p=P
                        ),
                    )

                    xt = xt_pool.tile([P, 2, CHUNK_TN], BF16)
                    xe_t = e_pool.tile([P, 2, CHUNK_TN], BF16, name="xe")
                    xe_strips[c] = xe_t
                    for dc in range(2):
                        pt = ps_t.tile(
                            [P, CHUNK_TN], F32, tag="pt", name=f"pt{dc}"
                        )
                        for s in range(4):
                            nc.tensor.transpose(
                                pt[:, s * P : (s + 1) * P],
                                xa[:, s, dc * P : (dc + 1) * P],
                                eye,
                            )
                        nc.scalar.activation(xt[:, dc, :], pt, AF.Copy)
                        e_t = e_pool.tile([P, CHUNK_TN], BF16, name="e")
                        nc.scalar.activation(e_t, pt, AF.Exp)
                        nc.vector.tensor_mul(xe_t[:, dc, :], pt, e_t)
                        nc.gpsimd.tensor_add(
                            acc_e[:, dc, :], acc_e[:, dc, :], e_t
                        )
                    # Two-chunk software pipeline keeps the PE off the
                    # ACT/DVE critical path of recent chunks.
                    pipe.append((c, xt))
                    if len(pipe) > 2:
                        stats_q_block(*pipe.pop(0))
                for ent in pipe:
                    stats_q_block(*ent)

                # ---------------- Phase B: TAtt.T, K.T, V.T, V
                sum_e = misc.tile([P, 2, N], F32)
                for dc in range(2):
                    nc.vector.tensor_add(
                        sum_e[:, dc, :],
                        acc_e[:, dc, 0:N],
                        acc_e[:, dc, N : 2 * N],
                    )
                rec = misc.tile([P, 2, N], F32)
                sxe_sb = misc.tile([P, 2, CHUNK_TN], F32)
                sum_xe = misc.tile([P, 2, N], F32)
                tatt_t = misc.tile([P, 2, N], BF16)  # TAtt.T [d, n] bf16
                for dc in range(2):
                    nc.vector.tensor_copy(sxe_sb[:, dc, :], acc_xe[dc])
                    nc.vector.tensor_add(
                        sum_xe[:, dc, :],
                        sxe_sb[:, dc, 0:N],
                        sxe_sb[:, dc, N : 2 * N],
                    )
                    nc.vector.reciprocal(rec[:, dc, :], sum_e[:, dc, :])
                    nc.vector.scalar_tensor_tensor(
                        out=tatt_t[:, dc, :],
                        in0=sum_xe[:, dc, :],
                        scalar=1.0,
                        in1=rec[:, dc, :],
                        op0=ALU.mult,
                        op1=ALU.mult,
                    )

                kt_sb = consts.tile([P, 2, N], BF16)  # K.T [j, m] (pre-scaled)
                vt2 = consts.tile([P, 2, 2, N], F32)  # V.T doubled per hg
                for w_sb, is_v in ((wkt_sb, 0), (wvt_sb, 1)):
                    for jc in range(2):
                        pk = ps_q.tile([P, N], F32, tag="pq", name="pk")
                        for kc in range(2):
                            nc.tensor.matmul(
                                pk,
                                w_sb[:, kc, jc * P : (jc + 1) * P],
                                tatt_t[:, kc, :],
                                start=(kc == 0),
                                stop=(kc == 1),
                            )
                        if not is_v:
                            nc.vector.tensor_copy(kt_sb[:, jc, :], pk)
                        else:
                            for ti in range(2):
                                nc.vector.tensor_copy(vt2[:, jc, ti, :], pk)

                v_sb = consts.tile([P, 2, D], BF16)  # V [m, j]
                for mc in range(2):
                    pv = ps_q.tile([P, D], F32, tag="pq", name="pv")
                    for kc in range(2):
                        nc.tensor.matmul(
                            pv,
                            tatt_t[:, kc, mc * P : (mc + 1) * P],
                            wvt_sb[:, kc, :],
                            start=(kc == 0),
                            stop=(kc == 1),
                        )
                    nc.scalar.activation(v_sb[:, mc, :], pv, AF.Copy)

            # ============ Phase C: attention + output ============
            with (
                tc.tile_pool(name="ps_a", bufs=3, space="PSUM") as ps_a,
                tc.tile_pool(name="ps_o", bufs=2, space="PSUM") as ps_o,
            ):
                # 5 ACT : 3 DVE relu-evac split.
                relu_acts = (0, 2, 4, 6, 7)

                def s_pair(c, k, a_str):
                    """S matmuls + relu evac for head-pair k of chunk c."""
                    hg, mc, rp = k >> 2, (k >> 1) & 1, k & 1
                    ps2 = ps_a.tile(
                        [P, 2 * CHUNK_TN], F32, tag="psa", name=f"ps{k}"
                    )
                    for rh in range(2):
                        r = rp * 2 + rh
                        nc.tensor.matmul(
                            ps2[:, rh * CHUNK_TN : (rh + 1) * CHUNK_TN],
                            kt_sb[
                                r * 32 : (r + 1) * 32, hg, mc * P : (mc + 1) * P
                            ],
                            qt_sb[
                                r * 32 : (r + 1) * 32,
                                hg,
                                c * CHUNK_TN : (c + 1) * CHUNK_TN,
                            ],
                            start=True,
                            stop=True,
                            tile_position=(r * 32, 0),
                        )
                    a2 = a_pool.tile(
                        [P, 2 * CHUNK_TN], BF16, tag="at", name=f"a{k}"
                    )
                    if k in relu_acts:
                        nc.scalar.activation(a2, ps2, AF.Relu)
                    else:
                        nc.vector.tensor_scalar_max(a2, ps2, 0.0)
                    for rh in range(2):
                        a_str[(hg, rp * 2 + rh, mc)] = a2[
                            :, rh * CHUNK_TN : (rh + 1) * CHUNK_TN
                        ]

                def av_pair(c, k, a_str, pos):
                    """A@V matmul pair k (of 8) for chunk c."""
                    for i in range(2):
                        j = 2 * k + i
                        hg, mc, r = j >> 3, (j >> 2) & 1, j & 3
                        if r == 0 and mc == 0:
                            pos[hg] = ps_o.tile(
                                [P, CHUNK_TN], F32, tag="po", name=f"po{hg}"
                            )
                        h = hg * 4 + r
                        nc.tensor.matmul(
                            pos[hg][r * 32 : (r + 1) * 32, :],
                            v_sb[:, mc, h * 32 : (h + 1) * 32],
                            a_str[(hg, r, mc)],
                            start=(mc == 0),
                            stop=(mc == 1),
                            tile_position=(0, r * 32),
                            skip_group_check=True,
                        )

                def po_evac(c, pos):
                    """+V fused evacuation of po, then DMA of out^T strip."""
                    for hg in range(2):
                        o_sb = o_pool.tile([P, CHUNK_TN], F32, name=f"o{hg}")
                        nc.vector.scalar_tensor_tensor(
                            out=o_sb,
                            in0=pos[hg],
                            scalar=1.0,
                            in1=vt2[:, hg, :, :],
                            op0=ALU.mult,
                            op1=ALU.add,
                        )
                        nc.sync.dma_start(
                            out=out_d[
                                hg * P : (hg + 1) * P,
                                c * CHUNK_TN : (c + 1) * CHUNK_TN,
                            ],
                            in_=o_sb,
                        )

                astrs = {}
                poss = {}
                for c in range(NCHUNKS + 2):
                    if c < NCHUNKS:
                        astrs[c] = {}
                    if c - 2 >= 0:
                        poss[c - 2] = {}
                    for kk in range(4):
                        for k in (2 * kk, 2 * kk + 1):
                            if c < NCHUNKS:
                                s_pair(c, k, astrs[c])
                        for k in (2 * kk, 2 * kk + 1):
                            if c - 2 >= 0:
                                av_pair(c - 2, k, astrs[c - 2], poss[c - 2])
                    if c - 2 >= 0:
                        po_evac(c - 2, poss.pop(c - 2))
                        astrs.pop(c - 2)

    nc.finalize()
    return nc


def prepare_in_maps(inputs):
    x = np.ascontiguousarray(np.asarray(inputs["x"], dtype=np.float32))
    w_q = np.asarray(inputs["W_Q"], dtype=np.float32)
    w_k = np.asarray(inputs["W_K"], dtype=np.float32)
    w_v = np.asarray(inputs["W_V"], dtype=np.float32)

    wqt = np.ascontiguousarray(w_q.T).astype(ml_dtypes.bfloat16)
    wkt = np.ascontiguousarray(w_k.T * np.float32(1.0 / np.sqrt(DH))).astype(
        ml_dtypes.bfloat16
    )
    wvt = np.ascontiguousarray(w_v.T).astype(ml_dtypes.bfloat16)

    return [
        {"x": np.ascontiguousarray(x[b]), "wqt": wqt, "wkt": wkt, "wvt": wvt}
        for b in range(B)
    ]


def kernel(**inputs) -> np.ndarray:
    if "nc" not in _CACHE:
        _CACHE["nc"] = _build_program()
    nc = _CACHE["nc"]

    in_maps = prepare_in_maps(inputs)
    res = run_bass_kernel_spmd(nc, in_maps, core_ids=list(range(B)))
    # Device emits out^T [D, T*N]; restore [T, N, D] during unshard.
    out = np.stack(
        [np.ascontiguousarray(res.results[b]["out"].T) for b in range(B)],
        axis=0,
    )
    return out.reshape(B, T, N, D)


# revision 21
# speedup vs baseline: 2.0848x; 1.0329x over previous
"""Trainium2 Bass kernel for nn_MultiHeadCDGCN (v4).

Math (per batch b):
  t_w  = softmax(x, axis=T);  TAtt = sum_T(x * t_w)          [N, D]
  Q    = x @ W_Q.T                                           [T, N, D]
  K    = TAtt @ W_K.T ; V = TAtt @ W_V.T                     [N, D]
  S_th = Q_th @ K_h.T / sqrt(dh)   (per t, head h)           [N, N]
  out  = (relu(S) + I) @ V = relu(S) @ V + V                 [T, N, D]

Sharding: data-parallel over B across 8 NeuronCores (B == 8, one batch
per core); no collectives. The device computes out^T [D, T*N] per
batch; the host unshard step restores [T, N, D] layout.

Structure:
  - S / A@V / Q matmuls in bf16 (fp32 matmuls lower to 2 HW passes;
    bf16 is 1 pass at 1 col/cycle). Softmax stats stay fp32-accurate:
    exp reads the fp32 x^T PSUM directly.
  - GPSIMD cannot touch PSUM on TRN2, so every PSUM evacuation is on
    ACT/DVE; GPSIMD owns the SBUF-only sum_e accumulation; sum_xe
    accumulates on the PE as identity-matmul PSUM accumulation (one
    PSUM bank per accumulation group - sharing a bank corrupts it).
  - Phase C interleaves S and A@V instruction pairs with a 2-chunk
    software pipeline so the PE never idles (idle gaps trigger HAM
    re-throttle; HW then runs matmuls below full clock).
  - +V is fused into the po evacuation as scalar_tensor_tensor.
"""

import sys

import numpy as np

sys.path.insert(0, "/opt/trn_rl_repo")

import ml_dtypes  # noqa: E402

import concourse.bacc as bacc  # noqa: E402
import concourse.tile as tile  # noqa: E402
from concourse import mybir  # noqa: E402
from concourse.masks import make_identity  # noqa: E402
from concourse.bass_utils import run_bass_kernel_spmd  # noqa: E402

F32 = mybir.dt.float32
BF16 = mybir.dt.bfloat16
AF = mybir.ActivationFunctionType
ALU = mybir.AluOpType

B, T, N, D, H, DH = 8, 32, 256, 256, 8, 32
P = 128
NCHUNKS = 16  # tn chunks of 512 (2 frames each)
CHUNK_T = 2
CHUNK_TN = CHUNK_T * N  # 512

_CACHE: dict = {}


def _build_program():
    nc = bacc.Bacc()

    x_d = nc.dram_tensor("x", [T, N, D], BF16, kind="ExternalInput")
    wqt_d = nc.dram_tensor("wqt", [D, D], BF16, kind="ExternalInput")
    wkt_d = nc.dram_tensor("wkt", [D, D], BF16, kind="ExternalInput")
    wvt_d = nc.dram_tensor("wvt", [D, D], BF16, kind="ExternalInput")
    out_d = nc.dram_tensor("out", [D, T * N], F32, kind="ExternalOutput")

    with tile.TileContext(nc) as tc:
        with (
            tc.tile_pool(name="consts", bufs=1) as consts,
            tc.tile_pool(name="xa", bufs=3) as xa_pool,
            tc.tile_pool(name="xt", bufs=3) as xt_pool,
            tc.tile_pool(name="ew", bufs=8) as e_pool,
            tc.tile_pool(name="at", bufs=32) as a_pool,
            tc.tile_pool(name="ot", bufs=4) as o_pool,
            tc.tile_pool(name="misc", bufs=1) as misc,
        ):
            eye = consts.tile([P, P], F32)
            make_identity(nc, eye)
            eye_bf = consts.tile([P, P], BF16)
            nc.vector.tensor_copy(eye_bf, eye)

            # Weights [k, j], k split over 2 partition tiles, bf16.
            # Issued on the scalar engine's DGE so they don't delay the x
            # chunk DMAs on sync.
            wqt_sb = consts.tile([P, 2, D], BF16)
            wkt_sb = consts.tile([P, 2, D], BF16)
            wvt_sb = consts.tile([P, 2, D], BF16)
            for w_sb, w_d in ((wqt_sb, wqt_d), (wkt_sb, wkt_d), (wvt_sb, wvt_d)):
                for kc in range(2):
                    nc.scalar.dma_start(
                        out=w_sb[:, kc, :],
                        in_=w_d[kc * P : (kc + 1) * P, :],
                    )

            # Q.T strip [j, tn] resident, bf16 (j split over 2 tiles).
            qt_sb = consts.tile([P, 2, T * N], BF16)

            # sum_e wide accumulator (SBUF, GPSIMD-owned).
            acc_e = consts.tile([P, 2, CHUNK_TN], F32)
            nc.gpsimd.memset(acc_e, 0.0)

            # ============ Phase A + B (stat PSUM pool scoped) ============
            with (
                tc.tile_pool(name="ps_t", bufs=3, space="PSUM") as ps_t,
                tc.tile_pool(name="ps_q", bufs=3, space="PSUM") as ps_q,
                tc.tile_pool(name="ps_s", bufs=1, space="PSUM") as ps_s,
            ):
                # sum_xe accumulators: one full PSUM bank per dc so the two
                # long-lived accumulation groups never share a bank.
                acc_xe_t = [
                    ps_s.tile([P, CHUNK_TN], F32, name=f"accxe{dc}")
                    for dc in range(2)
                ]
                acc_xe = {dc: acc_xe_t[dc] for dc in range(2)}

                xe_strips = {}

                def stats_q_block(c, xt):
                    """xe stat-acc + Q projection for chunk c (PE work)."""
                    xe_t = xe_strips.pop(c)
                    for dc in range(2):
                        nc.tensor.matmul(
                            acc_xe[dc],
                            eye_bf,
                            xe_t[:, dc, :],
                            start=(c == 0),
                            stop=(c == NCHUNKS - 1),
                            skip_group_check=True,
                        )
                    for jc in range(2):
                        pq = ps_q.tile(
                            [P, CHUNK_TN], F32, tag="pq", name=f"pq{jc}"
                        )
                        for kc in range(2):
                            nc.tensor.matmul(
                                pq,
                                wqt_sb[:, kc, jc * P : (jc + 1) * P],
                                xt[:, kc, :],
                                start=(kc == 0),
                                stop=(kc == 1),
                            )
                        dst = qt_sb[:, jc, c * CHUNK_TN : (c + 1) * CHUNK_TN]
                        if c >= NCHUNKS - 2:
                            nc.scalar.activation(dst, pq, AF.Copy)
                        else:
                            nc.vector.tensor_copy(dst, pq)

                pipe = []  # [(c, xt), ...]
                for c in range(NCHUNKS):
                    t0 = c * CHUNK_T
                    xa = xa_pool.tile([P, 4, D], BF16)
                    nc.sync.dma_start(
                        out=xa,
                        in_=x_d[t0 : t0 + CHUNK_T].rearrange(
                            "t (s p) d -> p (t s) d", p=P
                        ),
                    )

                    xt = xt_pool.tile([P, 2, CHUNK_TN], BF16)
                    xe_t = e_pool.tile([P, 2, CHUNK_TN], BF16, name="xe")
                    xe_strips[c] = xe_t
                    for dc in range(2):
                        pt = ps_t.tile(
                            [P, CHUNK_TN], BF16, tag="pt", name=f"pt{dc}"
                        )
                        for s in range(4):
                            nc.tensor.transpose(
                                pt[:, s * P : (s + 1) * P],
                                xa[:, s, dc * P : (dc + 1) * P],
                                eye_bf,
                            )
                        nc.scalar.activation(xt[:, dc, :], pt, AF.Copy)
                        e_t = e_pool.tile([P, CHUNK_TN], BF16, name="e")
                        nc.scalar.activation(e_t, pt, AF.Exp)
                        nc.vector.tensor_mul(xe_t[:, dc, :], pt, e_t)
                        nc.gpsimd.tensor_add(
                            acc_e[:, dc, :], acc_e[:, dc, :], e_t
                        )
                    # Two-chunk software pipeline keeps the PE off the
                    # ACT/DVE critical path of recent chunks.
                    pipe.append((c, xt))
                    if len(pipe) > 2:
                        stats_q_block(*pipe.pop(0))
                for ent in pipe:
                    stats_q_block(*ent)

                # ---------------- Phase B: TAtt.T, K.T, V.T, V
                sum_e = misc.tile([P, 2, N], F32)
                for dc in range(2):
                    nc.vector.tensor_add(
                        sum_e[:, dc, :],
                        acc_e[:, dc, 0:N],
                        acc_e[:, dc, N : 2 * N],
                    )
                rec = misc.tile([P, 2, N], F32)
                sxe_sb = misc.tile([P, 2, CHUNK_TN], F32)
                sum_xe = misc.tile([P, 2, N], F32)
                tatt_t = misc.tile([P, 2, N], BF16)  # TAtt.T [d, n] bf16
                for dc in range(2):
                    nc.vector.tensor_copy(sxe_sb[:, dc, :], acc_xe[dc])
                    nc.vector.tensor_add(
                        sum_xe[:, dc, :],
                        sxe_sb[:, dc, 0:N],
                        sxe_sb[:, dc, N : 2 * N],
                    )
                    nc.vector.reciprocal(rec[:, dc, :], sum_e[:, dc, :])
                    nc.vector.scalar_tensor_tensor(
                        out=tatt_t[:, dc, :],
                        in0=sum_xe[:, dc, :],
                        scalar=1.0,
                        in1=rec[:, dc, :],
                        op0=ALU.mult,
                        op1=ALU.mult,
                    )

                kt_sb = consts.tile([P, 2, N], BF16)  # K.T [j, m] (pre-scaled)
                vt2 = consts.tile([P, 2, 2, N], F32)  # V.T doubled per hg
                for w_sb, is_v in ((wkt_sb, 0), (wvt_sb, 1)):
                    for jc in range(2):
                        pk = ps_q.tile([P, N], F32, tag="pq", name="pk")
                        for kc in range(2):
                            nc.tensor.matmul(
                                pk,
                                w_sb[:, kc, jc * P : (jc + 1) * P],
                                tatt_t[:, kc, :],
                                start=(kc == 0),
                                stop=(kc == 1),
                            )
                        if not is_v:
                            nc.vector.tensor_copy(kt_sb[:, jc, :], pk)
                        else:
                            for ti in range(2):
                                nc.vector.tensor_copy(vt2[:, jc, ti, :], pk)

                v_sb = consts.tile([P, 2, D], BF16)  # V [m, j]
                for mc in range(2):
                    pv = ps_q.tile([P, D], F32, tag="pq", name="pv")
                    for kc in range(2):
                        nc.tensor.matmul(
                            pv,
                            tatt_t[:, kc, mc * P : (mc + 1) * P],
                            wvt_sb[:, kc, :],
                            start=(kc == 0),
                            stop=(kc == 1),
                        )
                    nc.scalar.activation(v_sb[:, mc, :], pv, AF.Copy)

            # ============ Phase C: attention + output ============
            with (
                tc.tile_pool(name="ps_a", bufs=3, space="PSUM") as ps_a,
                tc.tile_pool(name="ps_o", bufs=2, space="PSUM") as ps_o,
            ):
                # 5 ACT : 3 DVE relu-evac split.
                relu_acts = (0, 2, 4, 6, 7)

                def s_pair(c, k, a_str):
                    """S matmuls + relu evac for head-pair k of chunk c."""
                    hg, mc, rp = k >> 2, (k >> 1) & 1, k & 1
                    ps2 = ps_a.tile(
                        [P, 2 * CHUNK_TN], F32, tag="psa", name=f"ps{k}"
                    )
                    for rh in range(2):
                        r = rp * 2 + rh
                        nc.tensor.matmul(
                            ps2[:, rh * CHUNK_TN : (rh + 1) * CHUNK_TN],
                            kt_sb[
                                r * 32 : (r + 1) * 32, hg, mc * P : (mc + 1) * P
                            ],
                            qt_sb[
                                r * 32 : (r + 1) * 32,
                                hg,
                                c * CHUNK_TN : (c + 1) * CHUNK_TN,
                            ],
                            start=True,
                            stop=True,
                            tile_position=(r * 32, 0),
                        )
                    a2 = a_pool.tile(
                        [P, 2 * CHUNK_TN], BF16, tag="at", name=f"a{k}"
                    )
                    if k in relu_acts:
                        nc.scalar.activation(a2, ps2, AF.Relu)
                    else:
                        nc.vector.tensor_scalar_max(a2, ps2, 0.0)
                    for rh in range(2):
                        a_str[(hg, rp * 2 + rh, mc)] = a2[
                            :, rh * CHUNK_TN : (rh + 1) * CHUNK_TN
                        ]

                def av_pair(c, k, a_str, pos):
                    """A@V matmul pair k (of 8) for chunk c."""
                    for i in range(2):
                        j = 2 * k + i
                        hg, mc, r = j >> 3, (j >> 2) & 1, j & 3
                        if r == 0 and mc == 0:
                            pos[hg] = ps_o.tile(
                                [P, CHUNK_TN], F32, tag="po", name=f"po{hg}"
                            )
                        h = hg * 4 + r
                        nc.tensor.matmul(
                            pos[hg][r * 32 : (r + 1) * 32, :],
                            v_sb[:, mc, h * 32 : (h + 1) * 32],
                            a_str[(hg, r, mc)],
                            start=(mc == 0),
                            stop=(mc == 1),
                            tile_position=(0, r * 32),
                            skip_group_check=True,
                        )

                def po_evac(c, pos):
                    """+V fused evacuation of po, then DMA of out^T strip."""
                    for hg in range(2):
                        o_sb = o_pool.tile([P, CHUNK_TN], F32, name=f"o{hg}")
                        nc.vector.scalar_tensor_tensor(
                            out=o_sb,
                            in0=pos[hg],
                            scalar=1.0,
                            in1=vt2[:, hg, :, :],
                            op0=ALU.mult,
                            op1=ALU.add,
                        )
                        nc.sync.dma_start(
                            out=out_d[
                                hg * P : (hg + 1) * P,
                                c * CHUNK_TN : (c + 1) * CHUNK_TN,
                            ],
                            in_=o_sb,
                        )

                astrs = {}
                poss = {}
                for c in range(NCHUNKS + 2):
                    if c < NCHUNKS:
                        astrs[c] = {}
                    if c - 2 >= 0:
                        poss[c - 2] = {}
                    for kk in range(4):
                        for k in (2 * kk, 2 * kk + 1):
                            if c < NCHUNKS:
                                s_pair(c, k, astrs[c])
                        for k in (2 * kk, 2 * kk + 1):
                            if c - 2 >= 0:
                                av_pair(c - 2, k, astrs[c - 2], poss[c - 2])
                    if c - 2 >= 0:
                        po_evac(c - 2, poss.pop(c - 2))
                        astrs.pop(c - 2)

    nc.finalize()
    return nc


def prepare_in_maps(inputs):
    x = np.ascontiguousarray(np.asarray(inputs["x"], dtype=np.float32))
    w_q = np.asarray(inputs["W_Q"], dtype=np.float32)
    w_k = np.asarray(inputs["W_K"], dtype=np.float32)
    w_v = np.asarray(inputs["W_V"], dtype=np.float32)

    wqt = np.ascontiguousarray(w_q.T).astype(ml_dtypes.bfloat16)
    wkt = np.ascontiguousarray(w_k.T * np.float32(1.0 / np.sqrt(DH))).astype(
        ml_dtypes.bfloat16
    )
    wvt = np.ascontiguousarray(w_v.T).astype(ml_dtypes.bfloat16)

    xb = x.astype(ml_dtypes.bfloat16)
    return [
        {"x": np.ascontiguousarray(xb[b]), "wqt": wqt, "wkt": wkt, "wvt": wvt}
        for b in range(B)
    ]


def kernel(**inputs) -> np.ndarray:
    if "nc" not in _CACHE:
        _CACHE["nc"] = _build_program()
    nc = _CACHE["nc"]

    in_maps = prepare_in_maps(inputs)
    res = run_bass_kernel_spmd(nc, in_maps, core_ids=list(range(B)))
    # Device emits out^T [D, T*N]; restore [T, N, D] during unshard.
    out = np.stack(
        [np.ascontiguousarray(res.results[b]["out"].T) for b in range(B)],
        axis=0,
    )
    return out.reshape(B, T, N, D)
